# revision 8
# baseline (speedup 1.0000x reference)
"""Trainium2 Bass kernel for nn_Dense_EI (dense EI-masked MLP layer).

Math: out = scale * concat([x_exc, -4*x_inh], -1) @ bool_kernel
    = x @ K'  where K' = scale * kernel with inhibitory rows scaled by -4.

Device strategy (v2; baseline was 285.8us / rel err 1.33e-2):
  - fp8 e4m3 matmul with perf_mode=DoubleRow: on this HW a DR matmul
    retires 2 k-subtiles x 512 moving cols every ~216ns (2x bf16).
  - EI flip + the power-of-two part of `scale` fold into the kernel exactly
    (entries {0, 2^e, -2^(e+2)} are exact in e4m3); the mantissa part of
    scale folds into x.
  - fp8(x) alone gives rel err ~2.7e-2; appending an fp8 correction term
    (x - fp8(x)) for the 512 highest-nnz inhibitory rows gives 1.86e-2
    (< 2e-2 gate, deterministic: host fp8 emulation matches HW exactly)
    at K = 4096 + 512 = 4608 = 18*256 -> PE floor 18*64*216 = 249us.
  - Data-parallel over the 8 NeuronCores: each core computes 1024 of the
    8192 output rows.
  - v2 schedule (from baseline trace analysis: 23us over PE floor came
    from kxn-supply stalls at out-tile boundaries, a 14us epilogue of
    straggling output DMA + semaphore teardown, and 9.6us startup):
      * n-strip outer loop, m inner; kxm (x shard) fully cached in SBUF
        (4.6MB, fetched once as 6 chunk DMAs) -> kxn fetched once per
        strip instead of 15x: total DMA traffic 31MB vs 50MB.
      * kxn strips 1-7 land as ONE 2.3MB DMA each, triggered 2 strips
        (~66us) ahead: no per-tile trigger stream to stall behind.
        Strip 0 is 18 small per-k-tile DMAs so the first matmul can
        start as soon as k-tile 0 lands.
      * k innermost per m-subtile (18 accumulating matmuls per PSUM
        bank): evictions overlap the next bank's accumulation, and the
        output write for each m-subtile fires right after its evict, so
        the post-last-matmul tail is ~2us instead of 14us.
      * PE warmup bridges the NEFF-entry DMA latency so the HAM clock
        gate is released when real matmuls start.
"""

import sys

if "/opt/trn_rl_repo" not in sys.path:
    sys.path.insert(0, "/opt/trn_rl_repo")

import math

import ml_dtypes
import numpy as np

N_CORES = 8
B, S, IN_DIM, FEATURES = 4, 2048, 4096, 4096
M_TOTAL = B * S
P_EXC = 0.8
NUM_EXC = round(IN_DIM * P_EXC)  # 3277

P = 128
K_TILE = 256
MN = 512
N_CORR = 512  # inhibitory correction rows (by descending kernel-row nnz)
K2 = IN_DIM + N_CORR  # 4608
K_TILES = K2 // K_TILE  # 18
SUBS = K2 // P  # 36
M_CORE = M_TOTAL // N_CORES  # 1024
M_TILES = M_CORE // MN  # 2
M_SUB = MN // P  # 4
N_STRIPS = FEATURES // MN  # 8

# Startup chunk boundaries in k-tiles (shared by kxm m0/m1 and strip-0
# kxn): small chunks first so strip 0's all-m k-outer pass can start
# before the whole working set has landed.
CHUNKS = [(0, 2), (2, 4), (4, 9), (9, K_TILES)]

_module_cache: dict = {}

# Pre-compiled NEFF for the production shape (walrus output for the BIR this
# module builds; keyed by the BIR content hash). Seeding the compile cache
# with it skips the ~7 min walrus compile; any hash mismatch falls back to
# compiling from scratch.
_EMBEDDED_NEFFS: dict = {}


def _install_neff_cache():
    """Cache compiled NEFFs by BIR content hash (walrus compile is ~minutes)."""
    import hashlib
    import os
    import shutil

    from concourse import bass2jax

    if getattr(bass2jax, "_ant_neff_cache_installed", False):
        return
    orig = bass2jax.compile_bir_kernel
    cache_dir = os.environ.get("BASS_NEFF_CACHE", "/tmp/bass_neff_cache")
    os.makedirs(cache_dir, exist_ok=True)

    def cached(bir_json, tmpdir, neff_name="file.neff"):
        import re

        data = bir_json if isinstance(bir_json, bytes) else bir_json.encode()
        # The BIR's ant_debug annotations embed absolute file paths and call
        # tracebacks (directory- and caller-dependent); strip them from the
        # hash so the cache key depends only on the program.
        norm = re.sub(rb'"filename":"(?:[^"\\]|\\.)*"', b'"filename":""', data)
        norm = re.sub(
            rb'"ant_traceback":"(?:[^"\\]|\\.)*"', b'"ant_traceback":""', norm
        )
        norm = re.sub(rb'"lineno":\d+', b'"lineno":0', norm)
        h = hashlib.sha256(norm).hexdigest()[:24]
        path = os.path.join(cache_dir, f"{h}.neff")
        if not os.path.exists(path) and h in _EMBEDDED_NEFFS:
            import base64

            with open(path + f".tmp{os.getpid()}", "wb") as f:
                f.write(base64.b64decode(_EMBEDDED_NEFFS[h]))
            os.replace(path + f".tmp{os.getpid()}", path)
        if os.path.exists(path):
            dst = os.path.join(tmpdir, neff_name)
            shutil.copy(path, dst)
            return dst
        out = orig(bir_json, tmpdir, neff_name=neff_name)
        shutil.copy(out, path + f".tmp{os.getpid()}")
        os.replace(path + f".tmp{os.getpid()}", path)
        return out

    bass2jax.compile_bir_kernel = cached
    bass2jax._ant_neff_cache_installed = True


def _build_module():
    """Build + compile the per-core Bass module: mxn = kxm.T @ kxn.

    DRAM layouts (pre-tiled on host; k = sub*128 + p):
      kxm: [M_TILES, 128, SUBS, MN] fp8   (x shard, transposed)
      kxn: [N_STRIPS, 128, SUBS, MN] fp8  (EI/scale-folded kernel)
      mxn: [M_TILES, N_STRIPS, 128, M_SUB, MN] bf16
    """
    key = "v2"
    if key in _module_cache:
        return _module_cache[key]

    import concourse.bacc as bacc
    import concourse.mybir as mybir
    import concourse.tile as tile

    DR = mybir.MatmulPerfMode.DoubleRow

    # disable_frame_to_traceback: keeps caller file paths out of the BIR so
    # its content hash (NEFF cache key) is stable across directories/callers.
    nc = bacc.Bacc(
        "TRN2",
        target_bir_lowering=False,
        debug=False,
        disable_frame_to_traceback=True,
    )
    kxm = nc.dram_tensor(
        "kxm", [M_TILES, P, SUBS, MN], mybir.dt.float8e4, kind="ExternalInput"
    )
    kxn = nc.dram_tensor(
        "kxn", [N_STRIPS, P, SUBS, MN], mybir.dt.float8e4, kind="ExternalInput"
    )
    # bf16 output: halves the output HBM traffic; rounding adds ~1e-3 rel
    # err in quadrature - negligible vs 1.86e-2.
    mxn = nc.dram_tensor(
        "mxn",
        [M_TILES, N_STRIPS, P, M_SUB, MN],
        mybir.dt.bfloat16,
        kind="ExternalOutput",
    )
    with tile.TileContext(nc) as tc:
        # PE warmup: trivial matmuls run during the initial DMA wait so the
        # HAM clock gate releases (1.2 -> 2.4 GHz) before real matmuls.
        with (
            tc.tile_pool(name="warm", bufs=1) as warm,
            tc.tile_pool(name="warm_ps", bufs=1, space="PSUM") as warm_ps,
        ):
            wt = warm.tile([128, MN], mybir.dt.bfloat16)
            nc.vector.memset(wt[:], 0.0)
            wp = warm_ps.tile([128, MN], mybir.dt.float32)
            # Big moving dim (216ns each) keeps PE solidly busy for the HAM
            # activity monitor; sized to end ~when the first data lands.
            for _ in range(12):
                nc.tensor.matmul(wp[:], wt[:, :P], wt[:], start=True, stop=True)
        tc.swap_default_side()
        with (
            tc.tile_pool(name="kxm_pool", bufs=1) as kxm_pool,
            tc.tile_pool(name="kxn0_pool", bufs=1) as kxn0_pool,
            tc.tile_pool(name="kxns_pool", bufs=3) as kxns_pool,
            tc.tile_pool(name="out_pool", bufs=3) as out_pool,
            tc.tile_pool(name="psum_pool", bufs=8, space="PSUM") as psum_pool,
        ):
            kxm_t: dict = {}
            kxn0_t: dict = {}
            kxns_t: dict = {}

            def fetch_kxm(m, ci, eng):
                c0, c1 = CHUNKS[ci]
                t = kxm_pool.tile(
                    [P, 2 * (c1 - c0), MN],
                    mybir.dt.float8e4,
                    name=f"kxm_{m}_{ci}",
                    tag=f"kxm_{m}_{ci}",
                )
                eng.dma_start(t[:], kxm.ap()[m][:, 2 * c0 : 2 * c1, :])
                kxm_t[(m, ci)] = t

            def fetch_kxn0(ci, eng):
                c0, c1 = CHUNKS[ci]
                t = kxn0_pool.tile(
                    [P, 2 * (c1 - c0), MN],
                    mybir.dt.float8e4,
                    name=f"kxn0_{ci}",
                    tag=f"kxn0_{ci}",
                )
                eng.dma_start(t[:], kxn.ap()[0][:, 2 * c0 : 2 * c1, :])
                kxn0_t[ci] = t

            def fetch_strip(nidx, eng):
                t = kxns_pool.tile([P, SUBS, MN], mybir.dt.float8e4, name=f"kxns_{nidx}", tag="kxns")
                eng.dma_start(t[:], kxn.ap()[nidx])
                kxns_t[nidx] = t

            # First wave. Emission order doubles as scheduler priority and
            # as per-engine HWDGE FIFO order; chunk c's three tiles (kxn0,
            # kxm m0, kxm m1) travel together so strip 0's all-m k-outer
            # pass (3 fresh 128KB tiles per 8 matmuls) stays fed.
            fetch_kxn0(0, nc.sync)
            fetch_kxm(0, 0, nc.scalar)
            fetch_kxm(1, 0, nc.sync)
            fetch_kxn0(1, nc.scalar)
            fetch_kxm(0, 1, nc.sync)
            fetch_kxm(1, 1, nc.scalar)
            fetch_kxn0(2, nc.sync)
            fetch_kxm(0, 2, nc.scalar)
            fetch_kxm(1, 2, nc.sync)
            fetch_kxn0(3, nc.scalar)
            fetch_kxm(0, 3, nc.sync)
            fetch_kxm(1, 3, nc.scalar)
            fetch_strip(1, nc.sync)
            fetch_strip(2, nc.scalar)

            def chunk_of(j):
                for ci, (c0, c1) in enumerate(CHUNKS):
                    if j < c1:
                        return ci, j - c0
                raise AssertionError(j)

            def kxm_slice(m, j, ms):
                ci, o = chunk_of(j)
                t = kxm_t[(m, ci)]
                return t[:, 2 * o : 2 * o + 2, ms * P : (ms + 1) * P]

            def kxn_slice(nidx, j):
                if nidx == 0:
                    ci, o = chunk_of(j)
                    return kxn0_t[ci][:, 2 * o : 2 * o + 2, :]
                return kxns_t[nidx][:, 2 * j : 2 * j + 2, :]

            def evict_and_write(ot, ps_ms, m, nidx, ms, eng):
                nc.vector.tensor_copy(out=ot[:, ms : ms + 1, :], in_=ps_ms[:])
                eng.dma_start(
                    mxn.ap()[m, nidx][:, ms : ms + 1, :], ot[:, ms : ms + 1, :]
                )

            # Strip 0: k-outer over ALL m-subtiles of BOTH m-tiles (8
            # matmuls per k-tile, all 8 PSUM banks) - fresh-data demand is
            # 3 tiles per 1.73us, matched to the DMA ramp during startup.
            ot0 = [
                out_pool.tile([P, M_SUB, MN], mybir.dt.bfloat16, name=f"out_0_{m}", tag="out")
                for m in range(M_TILES)
            ]
            ps0 = [
                [
                    psum_pool.tile([P, MN], mybir.dt.float32, name=f"ps_0_{m}_{i}", tag="ps")
                    for i in range(M_SUB)
                ]
                for m in range(M_TILES)
            ]
            for j in range(K_TILES):
                for m in range(M_TILES):
                    for ms in range(M_SUB):
                        nc.tensor.matmul(
                            ps0[m][ms][:],
                            kxm_slice(m, j, ms),
                            kxn_slice(0, j),
                            start=(j == 0),
                            stop=(j == K_TILES - 1),
                            perf_mode=DR,
                        )
            for m in range(M_TILES):
                for ms in range(M_SUB):
                    evict_and_write(
                        ot0[m], ps0[m][ms], m, 0, ms,
                        nc.sync if m == 0 else nc.scalar,
                    )

            # Strips 1-7: k-inner per m-subtile - PSUM bank done after 18
            # matmuls; evict + output write overlap the next bank's
            # accumulation.
            for nidx in range(1, N_STRIPS):
                if nidx <= N_STRIPS - 3:
                    fetch_strip(nidx + 2, nc.sync if nidx % 2 else nc.scalar)
                for m in range(M_TILES):
                    ot = out_pool.tile([P, M_SUB, MN], mybir.dt.bfloat16, name=f"out_{nidx}_{m}", tag="out")
                    ps = [
                        psum_pool.tile([P, MN], mybir.dt.float32, name=f"ps_{nidx}_{m}_{i}", tag="ps")
                        for i in range(M_SUB)
                    ]
                    out_eng = nc.sync if (nidx * M_TILES + m) % 2 == 0 else nc.scalar
                    for ms in range(M_SUB):
                        for j in range(K_TILES):
                            nc.tensor.matmul(
                                ps[ms][:],
                                kxm_slice(m, j, ms),
                                kxn_slice(nidx, j),
                                start=(j == 0),
                                stop=(j == K_TILES - 1),
                                perf_mode=DR,
                            )
                        evict_and_write(ot, ps[ms], m, nidx, ms, out_eng)
    nc.compile()
    _module_cache[key] = nc
    return nc


def _prep_inputs(x_np: np.ndarray, kern_np: np.ndarray, scale_np: np.ndarray):
    """Host-side: EI/scale fold into kernel, hi/lo split of x, per-core shards."""
    in_dim = kern_np.shape[0]
    num_exc = round(in_dim * P_EXC)
    m_total = x_np.size // in_dim

    # Scale folding: s = m * 2^e with m in [1,2). The 2^e part goes into the
    # kernel - entries {2^e, -2^(e+2)} are powers of two, exact in e4m3 for
    # e >= -6 - and the mantissa part into x, keeping x's magnitude O(1),
    # clear of fp8 subnormals. For the production s = 1/64, m == 1 exactly.
    s = float(scale_np)
    e_exp = max(math.floor(math.log2(abs(s))), -6) if s != 0 else 0
    m_mant = s / (2.0**e_exp)

    kf = kern_np.astype(np.float32)
    ei = np.float32(-P_EXC / (1.0 - P_EXC))  # == -4.0 exactly in f32
    kf[num_exc:] *= ei
    kf *= np.float32(2.0**e_exp)
    kf8 = kf.astype(ml_dtypes.float8_e4m3)

    # Correct the N_CORR inhibitory rows with the most kernel nonzeros
    # (largest error weight): rel err 1.86e-2 at K = 4096+512 = 18*256.
    n_inh = in_dim - num_exc
    n_corr = min(n_inh, N_CORR)
    nnz = kern_np[num_exc:].astype(np.int32).sum(axis=1)
    sel = num_exc + np.argsort(-nnz)[:n_corr]

    # K layout: [full kernel (in_dim); selected inhibitory rows]
    k2 = in_dim + n_corr
    assert k2 == K2 and k2 % K_TILE == 0
    n = kf8.shape[1]
    kxn = np.empty((k2, n), dtype=ml_dtypes.float8_e4m3)
    kxn[:in_dim] = kf8
    kxn[in_dim:k2] = kf8[sel]

    xs = x_np.reshape(m_total, in_dim) * np.float32(m_mant)
    x8 = xs.astype(ml_dtypes.float8_e4m3)
    lo_sel = (xs[:, sel] - x8[:, sel].astype(np.float32)).astype(
        ml_dtypes.float8_e4m3
    )
    kxm_full = np.empty((k2, m_total), dtype=ml_dtypes.float8_e4m3)
    kxm_full[:in_dim] = x8.T
    kxm_full[in_dim:k2] = lo_sel.T

    # Pre-tile: [K2, 512] -> [128, SUBS, 512] with k = sub*128 + p.
    def tile_col(a):
        return a.reshape(SUBS, P, MN).transpose(1, 0, 2)

    kxn_t = np.ascontiguousarray(
        np.stack([tile_col(kxn[:, i * MN : (i + 1) * MN]) for i in range(N_STRIPS)])
    )
    kxm_shards = []
    for c in range(N_CORES):
        cs = kxm_full[:, c * M_CORE : (c + 1) * M_CORE]
        kxm_shards.append(
            np.ascontiguousarray(
                np.stack(
                    [tile_col(cs[:, mt * MN : (mt + 1) * MN]) for mt in range(M_TILES)]
                )
            )
        )
    return kxm_shards, kxn_t


def _run(x_np, kern_np, scale_np, trace=False, tmpdir=None):
    from concourse.bass_utils import run_bass_kernel_spmd

    _install_neff_cache()

    kxm_shards, kxn_t = _prep_inputs(x_np, kern_np, scale_np)
    nc = _build_module()

    in_maps = [{"kxm": kxm_shards[c], "kxn": kxn_t} for c in range(N_CORES)]
    # The axon-tunneled execute occasionally faults with
    # NRT_EXEC_UNIT_UNRECOVERABLE; a retry on a recovered device succeeds.
    last_err = None
    for attempt in range(4):
        try:
            res = run_bass_kernel_spmd(
                nc, in_maps, list(range(N_CORES)), trace=trace, tmpdir=tmpdir
            )
            # mxn is [M_TILES, N_STRIPS, 128, M_SUB, MN]:
            # out[mt*512 + ms*128 + p, nt*512 + j] = mxn[mt, nt, p, ms, j]
            outs = []
            for c in range(N_CORES):
                t = res.results[c]["mxn"]
                outs.append(
                    t.transpose(0, 3, 2, 1, 4)
                    .reshape(M_CORE, FEATURES)
                    .astype(np.float32)
                )
            out = np.concatenate(outs, axis=0)
            return out, res
        except Exception as err:  # noqa: BLE001
            # Retry runtime/device faults; deterministic build errors won't
            # heal, so re-raise those immediately.
            name = type(err).__name__
            retryable = "RuntimeError" in name or "Unavailable" in name
            if not retryable or attempt == 3:
                raise
            last_err = err
            import time as _time

            print(f"kernel attempt {attempt} failed: {err}", flush=True)
            _time.sleep(5.0 * (attempt + 1))
    raise last_err


def kernel(x, kernel, scale):
    x_np = np.asarray(x, dtype=np.float32)
    kern_np = np.asarray(kernel)
    scale_np = np.asarray(scale, dtype=np.float32)
    out, _ = _run(x_np, kern_np, scale_np)
    return out.reshape(x_np.shape[:-1] + (kern_np.shape[1],))


# revision 11
# speedup vs baseline: 1.0323x; 1.0323x over previous
"""Trainium2 Bass kernel for nn_Dense_EI (dense EI-masked MLP layer).

Math: out = scale * concat([x_exc, -4*x_inh], -1) @ bool_kernel
    = x @ K'  where K' = scale * kernel with inhibitory rows scaled by -4.

Device strategy (v2; baseline was 285.8us / rel err 1.33e-2):
  - fp8 e4m3 matmul with perf_mode=DoubleRow: on this HW a DR matmul
    retires 2 k-subtiles x 512 moving cols every ~216ns (2x bf16).
  - EI flip + the power-of-two part of `scale` fold into the kernel exactly
    (entries {0, 2^e, -2^(e+2)} are exact in e4m3); the mantissa part of
    scale folds into x.
  - fp8(x) alone gives rel err ~2.7e-2; appending an fp8 correction term
    (x - fp8(x)) for the 512 highest-nnz inhibitory rows gives 1.86e-2
    (< 2e-2 gate, deterministic: host fp8 emulation matches HW exactly)
    at K = 4096 + 512 = 4608 = 18*256 -> PE floor 18*64*216 = 249us.
  - Data-parallel over the 8 NeuronCores: each core computes 1024 of the
    8192 output rows.
  - v2 schedule (from baseline trace analysis: 23us over PE floor came
    from kxn-supply stalls at out-tile boundaries, a 14us epilogue of
    straggling output DMA + semaphore teardown, and 9.6us startup):
      * n-strip outer loop, m inner; kxm (x shard) fully cached in SBUF
        (4.6MB, fetched once as 6 chunk DMAs) -> kxn fetched once per
        strip instead of 15x: total DMA traffic 31MB vs 50MB.
      * kxn strips 1-7 land as ONE 2.3MB DMA each, triggered 2 strips
        (~66us) ahead: no per-tile trigger stream to stall behind.
        Strip 0 is 18 small per-k-tile DMAs so the first matmul can
        start as soon as k-tile 0 lands.
      * k innermost per m-subtile (18 accumulating matmuls per PSUM
        bank): evictions overlap the next bank's accumulation, and the
        output write for each m-subtile fires right after its evict, so
        the post-last-matmul tail is ~2us instead of 14us.
      * PE warmup bridges the NEFF-entry DMA latency so the HAM clock
        gate is released when real matmuls start.
"""

import sys

if "/opt/trn_rl_repo" not in sys.path:
    sys.path.insert(0, "/opt/trn_rl_repo")

import math

import ml_dtypes
import numpy as np

N_CORES = 8
B, S, IN_DIM, FEATURES = 4, 2048, 4096, 4096
M_TOTAL = B * S
P_EXC = 0.8
NUM_EXC = round(IN_DIM * P_EXC)  # 3277

P = 128
K_TILE = 256
MN = 512
N_CORR = 512  # inhibitory correction rows (by descending kernel-row nnz)
K2 = IN_DIM + N_CORR  # 4608
K_TILES = K2 // K_TILE  # 18
SUBS = K2 // P  # 36
M_CORE = M_TOTAL // N_CORES  # 1024
M_TILES = M_CORE // MN  # 2
M_SUB = MN // P  # 4
N_STRIPS = FEATURES // MN  # 8

# Startup chunk boundaries in k-tiles (shared by kxm m0/m1 and strip-0
# kxn): fine-grained so a matmul's all-or-nothing wait on its chunk tile
# tracks the DMA ramp closely (coarse chunks caused ~3us stalls that
# re-engaged the HAM clock gate).
CHUNKS = [(0, 2), (2, 4), (4, 6), (6, 9), (9, 12), (12, 15), (15, K_TILES)]

_module_cache: dict = {}

# Pre-compiled NEFF for the production shape (walrus output for the BIR this
# module builds; keyed by the BIR content hash). Seeding the compile cache
# with it skips the ~7 min walrus compile; any hash mismatch falls back to
# compiling from scratch.
_EMBEDDED_NEFFS: dict = {}


def _install_neff_cache():
    """Cache compiled NEFFs by BIR content hash (walrus compile is ~minutes)."""
    import hashlib
    import os
    import shutil

    from concourse import bass2jax

    if getattr(bass2jax, "_ant_neff_cache_installed", False):
        return
    orig = bass2jax.compile_bir_kernel
    cache_dir = os.environ.get("BASS_NEFF_CACHE", "/tmp/bass_neff_cache")
    os.makedirs(cache_dir, exist_ok=True)

    def cached(bir_json, tmpdir, neff_name="file.neff"):
        import re

        data = bir_json if isinstance(bir_json, bytes) else bir_json.encode()
        # The BIR's ant_debug annotations embed absolute file paths and call
        # tracebacks (directory- and caller-dependent); strip them from the
        # hash so the cache key depends only on the program.
        norm = re.sub(rb'"filename":"(?:[^"\\]|\\.)*"', b'"filename":""', data)
        norm = re.sub(
            rb'"ant_traceback":"(?:[^"\\]|\\.)*"', b'"ant_traceback":""', norm
        )
        norm = re.sub(rb'"lineno":\d+', b'"lineno":0', norm)
        h = hashlib.sha256(norm).hexdigest()[:24]
        path = os.path.join(cache_dir, f"{h}.neff")
        if not os.path.exists(path) and h in _EMBEDDED_NEFFS:
            import base64

            with open(path + f".tmp{os.getpid()}", "wb") as f:
                f.write(base64.b64decode(_EMBEDDED_NEFFS[h]))
            os.replace(path + f".tmp{os.getpid()}", path)
        if os.path.exists(path):
            dst = os.path.join(tmpdir, neff_name)
            shutil.copy(path, dst)
            return dst
        out = orig(bir_json, tmpdir, neff_name=neff_name)
        shutil.copy(out, path + f".tmp{os.getpid()}")
        os.replace(path + f".tmp{os.getpid()}", path)
        return out

    bass2jax.compile_bir_kernel = cached
    bass2jax._ant_neff_cache_installed = True


def _build_module():
    """Build + compile the per-core Bass module: mxn = kxm.T @ kxn.

    DRAM layouts (pre-tiled on host; k = sub*128 + p):
      kxm: [M_TILES, 128, SUBS, MN] fp8   (x shard, transposed)
      kxn: [N_STRIPS, 128, SUBS, MN] fp8  (EI/scale-folded kernel)
      mxn: [M_TILES, N_STRIPS, 128, M_SUB, MN] bf16
    """
    key = "v2"
    if key in _module_cache:
        return _module_cache[key]

    import concourse.bacc as bacc
    import concourse.mybir as mybir
    import concourse.tile as tile

    DR = mybir.MatmulPerfMode.DoubleRow

    # disable_frame_to_traceback: keeps caller file paths out of the BIR so
    # its content hash (NEFF cache key) is stable across directories/callers.
    nc = bacc.Bacc(
        "TRN2",
        target_bir_lowering=False,
        debug=False,
        disable_frame_to_traceback=True,
    )
    kxm = nc.dram_tensor(
        "kxm", [M_TILES, P, SUBS, MN], mybir.dt.float8e4, kind="ExternalInput"
    )
    kxn = nc.dram_tensor(
        "kxn", [N_STRIPS, P, SUBS, MN], mybir.dt.float8e4, kind="ExternalInput"
    )
    # bf16 output: halves the output HBM traffic; rounding adds ~1e-3 rel
    # err in quadrature - negligible vs 1.86e-2.
    mxn = nc.dram_tensor(
        "mxn",
        [M_TILES, N_STRIPS, P, M_SUB, MN],
        mybir.dt.bfloat16,
        kind="ExternalOutput",
    )
    with tile.TileContext(nc) as tc:
        # PE warmup: trivial matmuls run during the initial DMA wait so the
        # HAM clock gate releases (1.2 -> 2.4 GHz) before real matmuls.
        with (
            tc.tile_pool(name="warm", bufs=1) as warm,
            tc.tile_pool(name="warm_ps", bufs=1, space="PSUM") as warm_ps,
        ):
            wt = warm.tile([128, MN], mybir.dt.bfloat16)
            nc.vector.memset(wt[:], 0.0)
            wp = warm_ps.tile([128, MN], mybir.dt.float32)
            # Big moving dim (216ns each) keeps PE solidly busy for the HAM
            # activity monitor; sized to end ~when the first data lands.
            for _ in range(8):
                nc.tensor.matmul(wp[:], wt[:, :P], wt[:], start=True, stop=True)
        tc.swap_default_side()
        with (
            tc.tile_pool(name="kxm_pool", bufs=1) as kxm_pool,
            tc.tile_pool(name="kxn0_pool", bufs=1) as kxn0_pool,
            tc.tile_pool(name="kxns_pool", bufs=3) as kxns_pool,
            tc.tile_pool(name="out_pool", bufs=3) as out_pool,
            tc.tile_pool(name="psum_pool", bufs=8, space="PSUM") as psum_pool,
        ):
            kxm_t: dict = {}
            kxn0_t: dict = {}
            kxns_t: dict = {}

            def fetch_kxm(m, ci, eng):
                c0, c1 = CHUNKS[ci]
                t = kxm_pool.tile(
                    [P, 2 * (c1 - c0), MN],
                    mybir.dt.float8e4,
                    name=f"kxm_{m}_{ci}",
                    tag=f"kxm_{m}_{ci}",
                )
                eng.dma_start(t[:], kxm.ap()[m][:, 2 * c0 : 2 * c1, :])
                kxm_t[(m, ci)] = t

            def fetch_kxn0(ci, eng):
                c0, c1 = CHUNKS[ci]
                t = kxn0_pool.tile(
                    [P, 2 * (c1 - c0), MN],
                    mybir.dt.float8e4,
                    name=f"kxn0_{ci}",
                    tag=f"kxn0_{ci}",
                )
                eng.dma_start(t[:], kxn.ap()[0][:, 2 * c0 : 2 * c1, :])
                kxn0_t[ci] = t

            def fetch_strip(nidx, eng):
                t = kxns_pool.tile([P, SUBS, MN], mybir.dt.float8e4, name=f"kxns_{nidx}", tag="kxns")
                eng.dma_start(t[:], kxn.ap()[nidx])
                kxns_t[nidx] = t

            # First wave. Emission order doubles as scheduler priority and
            # as per-engine HWDGE FIFO order; chunk c's three tiles (kxn0,
            # kxm m0, kxm m1) travel together so strip 0's all-m k-outer
            # pass (3 fresh tiles per chunk of matmuls) stays fed.
            engs = [nc.sync, nc.scalar]
            t_i = 0
            for ci in range(len(CHUNKS)):
                fetch_kxn0(ci, engs[t_i % 2]); t_i += 1
                fetch_kxm(0, ci, engs[t_i % 2]); t_i += 1
                fetch_kxm(1, ci, engs[t_i % 2]); t_i += 1
            fetch_strip(1, nc.sync)
            fetch_strip(2, nc.scalar)

            def chunk_of(j):
                for ci, (c0, c1) in enumerate(CHUNKS):
                    if j < c1:
                        return ci, j - c0
                raise AssertionError(j)

            def kxm_slice(m, j, ms):
                ci, o = chunk_of(j)
                t = kxm_t[(m, ci)]
                return t[:, 2 * o : 2 * o + 2, ms * P : (ms + 1) * P]

            def kxn_slice(nidx, j):
                if nidx == 0:
                    ci, o = chunk_of(j)
                    return kxn0_t[ci][:, 2 * o : 2 * o + 2, :]
                return kxns_t[nidx][:, 2 * j : 2 * j + 2, :]

            def evict_and_write(ot, ps_ms, m, nidx, ms, eng):
                nc.vector.tensor_copy(out=ot[:, ms : ms + 1, :], in_=ps_ms[:])
                eng.dma_start(
                    mxn.ap()[m, nidx][:, ms : ms + 1, :], ot[:, ms : ms + 1, :]
                )

            # Strip 0: k-outer over ALL m-subtiles of BOTH m-tiles (8
            # matmuls per k-tile, all 8 PSUM banks) - fresh-data demand is
            # 3 tiles per 1.73us, matched to the DMA ramp during startup.
            ot0 = [
                out_pool.tile([P, M_SUB, MN], mybir.dt.bfloat16, name=f"out_0_{m}", tag="out")
                for m in range(M_TILES)
            ]
            ps0 = [
                [
                    psum_pool.tile([P, MN], mybir.dt.float32, name=f"ps_0_{m}_{i}", tag="ps")
                    for i in range(M_SUB)
                ]
                for m in range(M_TILES)
            ]
            for j in range(K_TILES):
                for m in range(M_TILES):
                    for ms in range(M_SUB):
                        nc.tensor.matmul(
                            ps0[m][ms][:],
                            kxm_slice(m, j, ms),
                            kxn_slice(0, j),
                            start=(j == 0),
                            stop=(j == K_TILES - 1),
                            perf_mode=DR,
                        )
            for m in range(M_TILES):
                for ms in range(M_SUB):
                    evict_and_write(
                        ot0[m], ps0[m][ms], m, 0, ms,
                        nc.sync if m == 0 else nc.scalar,
                    )

            # Strips 1-7: k-inner per m-subtile - PSUM bank done after 18
            # matmuls; evict + output write overlap the next bank's
            # accumulation.
            for nidx in range(1, N_STRIPS):
                if nidx <= N_STRIPS - 3:
                    fetch_strip(nidx + 2, nc.sync if nidx % 2 else nc.scalar)
                for m in range(M_TILES):
                    ot = out_pool.tile([P, M_SUB, MN], mybir.dt.bfloat16, name=f"out_{nidx}_{m}", tag="out")
                    ps = [
                        psum_pool.tile([P, MN], mybir.dt.float32, name=f"ps_{nidx}_{m}_{i}", tag="ps")
                        for i in range(M_SUB)
                    ]
                    out_eng = nc.sync if (nidx * M_TILES + m) % 2 == 0 else nc.scalar
                    for ms in range(M_SUB):
                        for j in range(K_TILES):
                            nc.tensor.matmul(
                                ps[ms][:],
                                kxm_slice(m, j, ms),
                                kxn_slice(nidx, j),
                                start=(j == 0),
                                stop=(j == K_TILES - 1),
                                perf_mode=DR,
                            )
                        evict_and_write(ot, ps[ms], m, nidx, ms, out_eng)
    nc.compile()
    _module_cache[key] = nc
    return nc


def _prep_inputs(x_np: np.ndarray, kern_np: np.ndarray, scale_np: np.ndarray):
    """Host-side: EI/scale fold into kernel, hi/lo split of x, per-core shards."""
    in_dim = kern_np.shape[0]
    num_exc = round(in_dim * P_EXC)
    m_total = x_np.size // in_dim

    # Scale folding: s = m * 2^e with m in [1,2). The 2^e part goes into the
    # kernel - entries {2^e, -2^(e+2)} are powers of two, exact in e4m3 for
    # e >= -6 - and the mantissa part into x, keeping x's magnitude O(1),
    # clear of fp8 subnormals. For the production s = 1/64, m == 1 exactly.
    s = float(scale_np)
    e_exp = max(math.floor(math.log2(abs(s))), -6) if s != 0 else 0
    m_mant = s / (2.0**e_exp)

    kf = kern_np.astype(np.float32)
    ei = np.float32(-P_EXC / (1.0 - P_EXC))  # == -4.0 exactly in f32
    kf[num_exc:] *= ei
    kf *= np.float32(2.0**e_exp)
    kf8 = kf.astype(ml_dtypes.float8_e4m3)

    # Correct the N_CORR inhibitory rows with the most kernel nonzeros
    # (largest error weight): rel err 1.86e-2 at K = 4096+512 = 18*256.
    n_inh = in_dim - num_exc
    n_corr = min(n_inh, N_CORR)
    nnz = kern_np[num_exc:].astype(np.int32).sum(axis=1)
    sel = num_exc + np.argsort(-nnz)[:n_corr]

    # K layout: [full kernel (in_dim); selected inhibitory rows]
    k2 = in_dim + n_corr
    assert k2 == K2 and k2 % K_TILE == 0
    n = kf8.shape[1]
    kxn = np.empty((k2, n), dtype=ml_dtypes.float8_e4m3)
    kxn[:in_dim] = kf8
    kxn[in_dim:k2] = kf8[sel]

    xs = x_np.reshape(m_total, in_dim) * np.float32(m_mant)
    x8 = xs.astype(ml_dtypes.float8_e4m3)
    lo_sel = (xs[:, sel] - x8[:, sel].astype(np.float32)).astype(
        ml_dtypes.float8_e4m3
    )
    kxm_full = np.empty((k2, m_total), dtype=ml_dtypes.float8_e4m3)
    kxm_full[:in_dim] = x8.T
    kxm_full[in_dim:k2] = lo_sel.T

    # Pre-tile: [K2, 512] -> [128, SUBS, 512] with k = sub*128 + p.
    def tile_col(a):
        return a.reshape(SUBS, P, MN).transpose(1, 0, 2)

    kxn_t = np.ascontiguousarray(
        np.stack([tile_col(kxn[:, i * MN : (i + 1) * MN]) for i in range(N_STRIPS)])
    )
    kxm_shards = []
    for c in range(N_CORES):
        cs = kxm_full[:, c * M_CORE : (c + 1) * M_CORE]
        kxm_shards.append(
            np.ascontiguousarray(
                np.stack(
                    [tile_col(cs[:, mt * MN : (mt + 1) * MN]) for mt in range(M_TILES)]
                )
            )
        )
    return kxm_shards, kxn_t


def _run(x_np, kern_np, scale_np, trace=False, tmpdir=None):
    from concourse.bass_utils import run_bass_kernel_spmd

    _install_neff_cache()

    kxm_shards, kxn_t = _prep_inputs(x_np, kern_np, scale_np)
    nc = _build_module()

    in_maps = [{"kxm": kxm_shards[c], "kxn": kxn_t} for c in range(N_CORES)]
    # The axon-tunneled execute occasionally faults with
    # NRT_EXEC_UNIT_UNRECOVERABLE; a retry on a recovered device succeeds.
    last_err = None
    for attempt in range(4):
        try:
            res = run_bass_kernel_spmd(
                nc, in_maps, list(range(N_CORES)), trace=trace, tmpdir=tmpdir
            )
            # mxn is [M_TILES, N_STRIPS, 128, M_SUB, MN]:
            # out[mt*512 + ms*128 + p, nt*512 + j] = mxn[mt, nt, p, ms, j]
            outs = []
            for c in range(N_CORES):
                t = res.results[c]["mxn"]
                outs.append(
                    t.transpose(0, 3, 2, 1, 4)
                    .reshape(M_CORE, FEATURES)
                    .astype(np.float32)
                )
            out = np.concatenate(outs, axis=0)
            return out, res
        except Exception as err:  # noqa: BLE001
            # Retry runtime/device faults; deterministic build errors won't
            # heal, so re-raise those immediately.
            name = type(err).__name__
            retryable = "RuntimeError" in name or "Unavailable" in name
            if not retryable or attempt == 3:
                raise
            last_err = err
            import time as _time

            print(f"kernel attempt {attempt} failed: {err}", flush=True)
            _time.sleep(5.0 * (attempt + 1))
    raise last_err


def kernel(x, kernel, scale):
    x_np = np.asarray(x, dtype=np.float32)
    kern_np = np.asarray(kernel)
    scale_np = np.asarray(scale, dtype=np.float32)
    out, _ = _run(x_np, kern_np, scale_np)
    return out.reshape(x_np.shape[:-1] + (kern_np.shape[1],))
